# revision 25
# baseline (speedup 1.0000x reference)
"""Trainium2 Bass kernel for nn_CrossFeatureAttention (4-layer post-norm
transformer encoder + accumulated mean attention map).

Data-parallel over batch: B=8 -> one batch element per NeuronCore.

Per-core layout:
  - activations feature-major: xT (d on partitions, t free), fp32
  - q,k feature-major bf16; v token-major bf16
  - scores (tq part, tk free) in PSUM; softmax via ACT Exp with accum_out
    row-sums; normalize via tensor_scalar(recip); A accumulated in bf16
    Apart tiles then folded into DRAM fp32 via gpsimd casting+accum DMA
  - attn@V: normalized attn bf16 DMA-xbar-transposed to (tk, tq); ctx
    feature-major with col-packed head pairs on the PE array
  - fp32 matmuls run as float32r (fp22, full PE rate)
  - LayerNorm over features (=partitions) via ones-matmul broadcast
"""
import numpy as np
import ml_dtypes

L, D, H, FF = 4, 512, 8, 2048
HD = D // H          # 64
T = 1024
B = 8
P = 128
NT = T // P          # 8 token tiles
ND = D // P          # 4 feature tiles
NF2 = (2 * D) // P   # 8 q+k feature tiles
NFF = FF // P        # 16 ffn tiles
EPS = 1e-5
INV_SQRT_HD = 0.125
NCORES = 8

_CACHE = {}
DEBUG_STAGES = False


def _build(nl=L):
    import concourse.bass as bass
    import concourse.mybir as mybir
    import concourse.tile as tile
    from concourse import bacc

    f32 = mybir.dt.float32
    f32r = mybir.dt.float32r
    bf16 = mybir.dt.bfloat16
    Alu = mybir.AluOpType
    Act = mybir.ActivationFunctionType

    nc = bacc.Bacc("TRN2", target_bir_lowering=False, debug=False,
                   num_devices=NCORES)

    ET = nc.declare_dram_parameter("ET", [D, T], f32, isOutput=False)
    WQKT = nc.declare_dram_parameter("WQKT", [nl, D, 2 * D], f32, isOutput=False)
    WVT = nc.declare_dram_parameter("WVT", [nl, D, D], f32, isOutput=False)
    WOT = nc.declare_dram_parameter("WOT", [nl, D, D], f32, isOutput=False)
    W1T = nc.declare_dram_parameter("W1T", [nl, D, FF], f32, isOutput=False)
    W2T = nc.declare_dram_parameter("W2T", [nl, FF, D], bf16, isOutput=False)
    # cols: 0:8 bqk | 8:12 bo | 12:28 b1 | 28:32 b2 | 32:36 g1 | 36:40 be1
    #       | 40:44 g2 | 44:48 be2
    BC = nc.declare_dram_parameter("BC", [P, nl, 48], f32, isOutput=False)
    BV = nc.declare_dram_parameter("BV", [nl, D], f32, isOutput=False)
    ONED = nc.declare_dram_parameter("ONED", [P, P], f32, isOutput=False)
    ONER = nc.declare_dram_parameter("ONER", [1, P], f32, isOutput=False)

    XOUT = nc.declare_dram_parameter("XOUT", [D, T], f32, isOutput=True)
    AOUT = nc.declare_dram_parameter("AOUT", [T, T], f32, isOutput=True)
    DBG = (nc.declare_dram_parameter("DBG", [5, D, T], f32, isOutput=True)
           if DEBUG_STAGES else None)
    DBG2 = (nc.declare_dram_parameter("DBG2", [T, T], bf16, isOutput=True)
            if DEBUG_STAGES else None)

    def dump(slot, tiles):
        if DBG is None:
            return
        for dt_, t_ in enumerate(tiles):
            nc.sync.dma_start(
                DBG[slot, dt_ * P:(dt_ + 1) * P, :].bitcast(t_.dtype), t_[:])

    def ln(xr, l, gcol, bcol, px, ptmp, pstat, pA, onesD, bc, zc, ec):
        """LayerNorm over partitions of 4 feature-major [128,T] f32 tiles."""
        x2 = []
        for dt in range(ND):
            t2 = ptmp.tile([P, T], f32r, tag="ptmp")
            nc.scalar.activation(t2[:], xr[dt][:], Act.Square, bias=zc[:])
            x2.append(t2)
        mean_ps = pA.tile([P, T], f32, tag="pA")
        msq_ps = pA.tile([P, T], f32, tag="pA")
        for ch in range(2):
            sl = slice(ch * 512, (ch + 1) * 512)
            for dt in range(ND):
                nc.tensor.matmul(mean_ps[:, sl], onesD[:],
                                 xr[dt][:, sl].bitcast(f32r),
                                 start=(dt == 0), stop=(dt == ND - 1))
            for dt in range(ND):
                nc.tensor.matmul(msq_ps[:, sl], onesD[:],
                                 x2[dt][:, sl].bitcast(f32r),
                                 start=(dt == 0), stop=(dt == ND - 1))
        mean_sb = pstat.tile([P, T], f32, tag="stat")
        nc.scalar.copy(mean_sb[:], mean_ps[:])
        m2 = pstat.tile([P, T], f32, tag="stat")
        nc.vector.tensor_tensor(m2[:], mean_sb[:], mean_sb[:], Alu.mult)
        var = pstat.tile([P, T], f32, tag="stat")
        nc.vector.tensor_tensor(var[:], msq_ps[:], m2[:], Alu.subtract)
        std = pstat.tile([P, T], f32, tag="stat")
        nc.scalar.activation(std[:], var[:], Act.Sqrt, bias=ec[:])
        rstd = pstat.tile([P, T], f32, tag="stat")
        nc.vector.reciprocal(rstd[:], std[:])
        out = []
        for dt in range(ND):
            tsub = ptmp.tile([P, T], f32, tag="ptmp")
            nc.vector.tensor_tensor(tsub[:], xr[dt][:], mean_sb[:], Alu.subtract)
            xo = px.tile([P, T], f32r, tag="x")
            nc.vector.tensor_tensor(xo[:], tsub[:], rstd[:], Alu.mult)
            xo2 = px.tile([P, T], f32r, tag="x")
            nc.vector.tensor_scalar(xo2[:], xo[:],
                                    bc[:, l, gcol + dt:gcol + dt + 1],
                                    bc[:, l, bcol + dt:bcol + dt + 1],
                                    Alu.mult, Alu.add)
            out.append(xo2)
        return out

    with tile.TileContext(nc) as tc:
        with (
            tc.tile_pool(name="px", bufs=11) as px,
            tc.tile_pool(name="ptmp", bufs=5) as ptmp,
            tc.tile_pool(name="pqk", bufs=NF2) as pqk,
            tc.tile_pool(name="pv", bufs=NT) as pv,
            tc.tile_pool(name="pattn", bufs=4) as pattn,
            tc.tile_pool(name="pat", bufs=2) as pat,
            tc.tile_pool(name="papart", bufs=4) as papart,
            tc.tile_pool(name="pw", bufs=2) as pw,
            tc.tile_pool(name="pw2", bufs=2) as pw2,
            tc.tile_pool(name="psmall", bufs=4) as psmall,
            tc.tile_pool(name="pconst", bufs=1) as pconst,
            tc.tile_pool(name="pstat", bufs=4) as pstat,
            tc.tile_pool(name="pA", bufs=3, space="PSUM") as pA,
            tc.tile_pool(name="pB", bufs=2, space="PSUM") as pB,
        ):
            ones_row = pconst.tile([1, P], f32r, tag="ones_row")
            nc.sync.dma_start(ones_row[:], ONER[:].bitcast(f32r))
            zero_col = pconst.tile([P, 1], f32, tag="zero_col")
            nc.vector.memset(zero_col[:], 0.0)
            eps_col = pconst.tile([P, 1], f32, tag="eps_col")
            nc.vector.memset(eps_col[:], EPS)
            onesD = pconst.tile([P, P], f32r, tag="onesD")
            nc.sync.dma_start(onesD[:], ONED[:].bitcast(f32r))
            bc = pconst.tile([P, nl, 48], f32, tag="bc")
            nc.sync.dma_start(bc[:], BC[:])

            x = []
            for dt in range(ND):
                t_ = px.tile([P, T], f32r, tag="x")
                nc.sync.dma_start(t_[:], ET[dt * P:(dt + 1) * P, :].bitcast(f32r))
                x.append(t_)

            for l in range(nl):
                bias = lambda c: bc[:, l, c:c + 1]

                # ---------------- QKV ----------------
                qk = []
                for ft in range(NF2):
                    wqk = pw.tile([P, ND, P], f32r, tag="wqk")
                    nc.sync.dma_start(
                        wqk[:],
                        WQKT[l, :, ft * P:(ft + 1) * P].rearrange(
                            "(a p) c -> p a c", p=P).bitcast(f32r))
                    ps = pA.tile([P, T], f32, tag="pA")
                    for ch in range(2):
                        sl = slice(ch * 512, (ch + 1) * 512)
                        for dt in range(ND):
                            nc.tensor.matmul(ps[:, sl], wqk[:, dt, :],
                                             x[dt][:, sl].bitcast(f32r),
                                             start=(dt == 0),
                                             stop=(dt == ND - 1))
                    qt = pqk.tile([P, T], bf16, tag="qk")
                    nc.scalar.activation(qt[:], ps[:], Act.Identity,
                                         bias=bias(ft))
                    qk.append(qt)

                bv = psmall.tile([1, D], f32r, tag="bv")
                nc.sync.dma_start(bv[:], BV[l:l + 1, :].bitcast(f32r))
                wv = pw.tile([P, ND, D], f32r, tag="wv", bufs=1)
                nc.sync.dma_start(
                    wv[:], WVT[l].rearrange("(a p) c -> p a c", p=P).bitcast(f32r))
                v = []
                for tt in range(NT):
                    ps = pB.tile([P, D], f32, tag="pB")
                    for dt in range(ND):
                        nc.tensor.matmul(
                            ps[:], x[dt][:, tt * P:(tt + 1) * P].bitcast(f32r),
                            wv[:, dt, :], start=(dt == 0), stop=False)
                    nc.tensor.matmul(ps[:], ones_row[:], bv[:],
                                     start=False, stop=True)
                    vt = pv.tile([P, D], bf16, tag="v")
                    nc.scalar.copy(vt[:], ps[:])
                    v.append(vt)

                # ---------------- attention ----------------
                ctxs = []
                for hp in range(4):
                    h0, h1 = 2 * hp, 2 * hp + 1
                    qt, kt = qk[hp], qk[4 + hp]
                    ath = {h: pat.tile([P, NT, T], bf16, tag="at",
                                       name=f"ath{l}_{h}")
                           for h in (h0, h1)}
                    for tqt in range(NT):
                        apart = papart.tile([P, T], bf16, tag="apart")
                        den = psmall.tile([P, 2], f32, tag="den")
                        an = {}
                        for hi, h in enumerate((h0, h1)):
                            rb = hi * HD
                            ps = pA.tile([P, T], f32, tag="pA")
                            for ch in range(2):
                                sl = slice(ch * 512, (ch + 1) * 512)
                                nc.tensor.matmul(
                                    ps[:, sl],
                                    qt[rb:rb + HD, tqt * P:(tqt + 1) * P],
                                    kt[rb:rb + HD, sl],
                                    start=True, stop=True)
                            at = pattn.tile([P, T], bf16, tag="attn")
                            nc.scalar.activation(at[:], ps[:], Act.Exp,
                                                 bias=zero_col[:],
                                                 scale=INV_SQRT_HD,
                                                 accum_out=den[:, hi:hi + 1])
                            an[h] = at
                        rden = psmall.tile([P, 2], f32, tag="rden")
                        nc.vector.reciprocal(rden[:], den[:])
                        for hi, h in enumerate((h0, h1)):
                            attn_n = pattn.tile([P, T], bf16, tag="attn")
                            nc.vector.tensor_scalar(
                                attn_n[:], an[h][:], rden[:, hi:hi + 1], None,
                                Alu.mult)
                            if hi == 0:
                                nc.vector.tensor_copy(apart[:], attn_n[:])
                            else:
                                nc.vector.tensor_tensor(
                                    apart[:], attn_n[:], apart[:], Alu.add)
                            nc.scalar.dma_start(
                                ath[h][:, :, tqt * P:(tqt + 1) * P],
                                attn_n[:], transpose=True)
                        nc.gpsimd.dma_start(
                            AOUT[tqt * P:(tqt + 1) * P, :], apart[:],
                            accum_op=(Alu.bypass if (l == 0 and hp == 0)
                                      else Alu.add))
                    ctx = ptmp.tile([P, T], f32r, tag="ptmp")
                    for ch in range(2):
                        sl = slice(ch * 512, (ch + 1) * 512)
                        cps = pB.tile([P, 512], f32, tag="pB")
                        for hi, h in enumerate((h0, h1)):
                            for tkt in range(NT):
                                nc.tensor.matmul(
                                    cps[hi * HD:(hi + 1) * HD, :],
                                    v[tkt][:, h * HD:(h + 1) * HD],
                                    ath[h][:, tkt, sl],
                                    start=(tkt == 0), stop=(tkt == NT - 1),
                                    tile_position=(0, hi * HD))
                        nc.scalar.copy(ctx[:, sl], cps[:])
                    if l == 0 and hp == 0 and DBG2 is not None:
                        for tkt in range(NT):
                            nc.sync.dma_start(
                                DBG2[tkt * P:(tkt + 1) * P, :],
                                ath[0][:, tkt, :])
                    ctxs.append(ctx)
                if l == 0:
                    dump(0, ctxs)

                # ---------------- out-proj + residual + LN1 ----------------
                wo = pw.tile([P, ND, D], f32r, tag="wo", bufs=1)
                nc.sync.dma_start(
                    wo[:], WOT[l].rearrange("(a p) c -> p a c", p=P).bitcast(f32r))
                xr = []
                for ot in range(ND):
                    ps = pA.tile([P, T], f32, tag="pA")
                    for ch in range(2):
                        sl = slice(ch * 512, (ch + 1) * 512)
                        for ft in range(ND):
                            nc.tensor.matmul(
                                ps[:, sl], wo[:, ft, ot * P:(ot + 1) * P],
                                ctxs[ft][:, sl].bitcast(f32r),
                                start=(ft == 0), stop=(ft == ND - 1))
                    xn = px.tile([P, T], f32r, tag="x")
                    nc.vector.scalar_tensor_tensor(
                        xn[:], ps[:], bias(8 + ot), x[ot][:],
                        Alu.add, Alu.add)
                    xr.append(xn)
                if l == 0:
                    dump(1, xr)
                x = ln(xr, l, 32, 36, px, ptmp, pstat, pA, onesD, bc,
                       zero_col, eps_col)
                if l == 0:
                    dump(2, x)

                # ---------------- FFN ----------------
                h1big = [pat.tile([P, NT, T], bf16, tag="at",
                                  name=f"h1big{l}_{i}") for i in range(2)]
                h1s = []
                for fft in range(NFF):
                    w1 = pw.tile([P, ND, P], f32r, tag="w1")
                    nc.sync.dma_start(
                        w1[:],
                        W1T[l, :, fft * P:(fft + 1) * P].rearrange(
                            "(a p) c -> p a c", p=P).bitcast(f32r))
                    ps = pA.tile([P, T], f32, tag="pA")
                    for ch in range(2):
                        sl = slice(ch * 512, (ch + 1) * 512)
                        for dt in range(ND):
                            nc.tensor.matmul(ps[:, sl], w1[:, dt, :],
                                             x[dt][:, sl].bitcast(f32r),
                                             start=(dt == 0),
                                             stop=(dt == ND - 1))
                    ht = h1big[fft // NT][:, fft % NT, :]
                    nc.scalar.activation(ht, ps[:], Act.Relu,
                                         bias=bias(12 + fft))
                    h1s.append(ht)
                xr2 = []
                for ot in range(ND):
                    w2 = pw2.tile([P, NFF, P], bf16, tag="w2")
                    nc.sync.dma_start(
                        w2[:], W2T[l, :, ot * P:(ot + 1) * P].rearrange(
                            "(a p) c -> p a c", p=P))
                    ps = pA.tile([P, T], f32, tag="pA")
                    for fft in range(NFF):
                        for ch in range(2):
                            sl = slice(ch * 512, (ch + 1) * 512)
                            nc.tensor.matmul(ps[:, sl], w2[:, fft, :],
                                             h1s[fft][:, sl],
                                             start=(fft == 0),
                                             stop=(fft == NFF - 1))
                    xn = px.tile([P, T], f32r, tag="x")
                    nc.vector.scalar_tensor_tensor(
                        xn[:], ps[:], bias(28 + ot), x[ot][:],
                        Alu.add, Alu.add)
                    xr2.append(xn)
                if l == 0:
                    dump(3, xr2)
                x = ln(xr2, l, 40, 44, px, ptmp, pstat, pA, onesD, bc,
                       zero_col, eps_col)
                if l == 0:
                    dump(4, x)

            for dt in range(ND):
                nc.sync.dma_start(XOUT[dt * P:(dt + 1) * P, :].bitcast(f32r),
                                  x[dt][:])
    nc.compile()
    return nc


def _prep_inputs(Wqkv, bqkv, Wo, bo, W1, b1, W2, b2, g1, be1, g2, be2, nl):
    """Host-side packing. Returns the per-core-shared weight map."""
    f = np.float32
    WQKT = np.ascontiguousarray(
        np.transpose(Wqkv[:nl, :2 * D, :], (0, 2, 1)).astype(f))   # (nl,D,2D)
    WVT = np.ascontiguousarray(
        np.transpose(Wqkv[:nl, 2 * D:, :], (0, 2, 1)).astype(f))   # (nl,D,D)
    WOT = np.ascontiguousarray(np.transpose(Wo[:nl], (0, 2, 1)).astype(f))
    W1T = np.ascontiguousarray(np.transpose(W1[:nl], (0, 2, 1)).astype(f))
    W2T = np.ascontiguousarray(
        np.transpose(W2[:nl], (0, 2, 1)).astype(ml_dtypes.bfloat16))
    BC = np.zeros((P, nl, 48), f)
    for l in range(nl):
        BC[:, l, 0:8] = bqkv[l, :2 * D].reshape(8, P).T
        BC[:, l, 8:12] = bo[l].reshape(4, P).T
        BC[:, l, 12:28] = b1[l].reshape(16, P).T
        BC[:, l, 28:32] = b2[l].reshape(4, P).T
        BC[:, l, 32:36] = g1[l].reshape(4, P).T
        BC[:, l, 36:40] = be1[l].reshape(4, P).T
        BC[:, l, 40:44] = g2[l].reshape(4, P).T
        BC[:, l, 44:48] = be2[l].reshape(4, P).T
    BV = np.ascontiguousarray(bqkv[:nl, 2 * D:]).astype(f)
    return {"WQKT": WQKT, "WVT": WVT, "WOT": WOT, "W1T": W1T, "W2T": W2T,
            "BC": BC, "BV": BV,
            "ONED": np.full((P, P), 1.0 / D, f),
            "ONER": np.ones((1, P), f)}


def kernel(E, Wqkv, bqkv, Wo, bo, W1, b1, W2, b2, g1, be1, g2, be2,
           _nl=L, _return_res=False):
    from concourse.bass_utils import run_bass_kernel_spmd

    E = np.asarray(E, np.float32)
    if _nl not in _CACHE:
        _CACHE[_nl] = _build(_nl)
    nc = _CACHE[_nl]

    shared = _prep_inputs(np.asarray(Wqkv), np.asarray(bqkv), np.asarray(Wo),
                          np.asarray(bo), np.asarray(W1), np.asarray(b1),
                          np.asarray(W2), np.asarray(b2), np.asarray(g1),
                          np.asarray(be1), np.asarray(g2), np.asarray(be2),
                          _nl)
    in_maps = []
    for b in range(B):
        m = dict(shared)
        m["ET"] = np.ascontiguousarray(E[b].T)   # (D, T)
        in_maps.append(m)

    res = run_bass_kernel_spmd(nc, in_maps, list(range(NCORES)))
    xs, As = [], []
    for b in range(B):
        xs.append(res.results[b]["XOUT"].T)                  # (T, D)
        As.append(res.results[b]["AOUT"] * (1.0 / (_nl * H)))
    out_x = np.stack(xs).astype(np.float32)
    out_A = np.stack(As).astype(np.float32)
    if _return_res:
        return (out_x, out_A), res
    return out_x, out_A
